# revision 16
# baseline (speedup 1.0000x reference)
"""Trainium2 Bass kernel for windowed-causal multi-head attention w/ RoPE + KV cache.

Problem: nn_MultiHeadAttention_42460046688339
  x [16,256,2048], past_k/v [16,32,512,64], Wq/Wk/Wv/Wo [2048,2048], pos_offset=512
  returns (out [16,256,2048], new_k [16,32,512,64], new_v [16,32,512,64])

Sharding: data-parallel over batch across 8 cores (2 batches/core), no collectives.

Per-core design (fp32r matmuls, feature-major attention):
  - x -> xT via PE transposes; weights streamed natural-layout and PE-transposed
    per 128x128 tile (fp32 has no DMA-transpose path on trn2).
  - q/k projections token-major; RoPE on the free dim; unrotated cur_k/cur_v
    DMA'd to new_k/new_v; past halves of the caches are DRAM->DRAM copies.
  - scores computed transposed ([s, t]) so softmax needs no cross-partition
    reduction: exp on ACT with the 1/sqrt(64) fold into its scale, the
    windowed-causal mask as a 0/1 multiply on the two partially-masked
    s-tiles, and the denominator via an appended ones-column on v inside the
    attn@v matmul. Normalization = DVE reciprocal + GpSimd partition
    broadcast + DVE multiply.
"""

import numpy as np

B, T, DMODEL = 16, 256, 2048
H, HD = 32, 64
CACHE, WINDOW = 512, 512
THETA = 10000.0
NCORES = 8
BLOC = B // NCORES           # 2 batches per core
TOK = BLOC * T               # 512 tokens per core
NK = DMODEL // 128           # 16 contraction tiles
NCH = 8                      # 8 output-feature chunks of 256
PASTVIS = CACHE - (WINDOW - T)   # 256 visible past positions

_cache = {}


def _build_nc():
    import contextlib

    import concourse.mybir as mybir
    import concourse.tile as tile
    from concourse import bacc
    from concourse.masks import make_identity

    f32 = mybir.dt.float32
    f32r = mybir.dt.float32r
    AF = mybir.ActivationFunctionType
    OP = mybir.AluOpType

    nc = bacc.Bacc("TRN2", target_bir_lowering=False, debug=False, num_devices=1)

    x_d = nc.dram_tensor("x", [TOK, DMODEL], f32, kind="ExternalInput")
    pk_d = nc.dram_tensor("past_k", [BLOC, H, CACHE, HD], f32, kind="ExternalInput")
    pv_d = nc.dram_tensor("past_v", [BLOC, H, CACHE, HD], f32, kind="ExternalInput")
    w_d = {
        n: nc.dram_tensor(n, [DMODEL, DMODEL], f32, kind="ExternalInput")
        for n in ["Wq", "Wk", "Wv", "Wo"]
    }
    cos_d = nc.dram_tensor("cos_t", [128, 2, HD], f32, kind="ExternalInput")
    sin_d = nc.dram_tensor("sinsgn_t", [128, 2, HD], f32, kind="ExternalInput")
    mask_d = nc.dram_tensor("mask_t", [128, 2, T], f32, kind="ExternalInput")

    out_d = nc.dram_tensor("out", [TOK, DMODEL], f32, kind="ExternalOutput")
    nk_d = nc.dram_tensor("new_k", [BLOC, H, WINDOW, HD], f32, kind="ExternalOutput")
    nv_d = nc.dram_tensor("new_v", [BLOC, H, WINDOW, HD], f32, kind="ExternalOutput")

    with tile.TileContext(nc) as tc, contextlib.ExitStack() as ctx:
        const = ctx.enter_context(tc.tile_pool(name="const", bufs=1))
        sb = ctx.enter_context(tc.tile_pool(name="sb", bufs=1))
        big = ctx.enter_context(tc.tile_pool(name="big", bufs=1))
        wrp = ctx.enter_context(tc.tile_pool(name="wrows", bufs=1))
        wtp = ctx.enter_context(tc.tile_pool(name="wT", bufs=1))
        tmp = ctx.enter_context(tc.tile_pool(name="ropetmp", bufs=1))
        pp = ctx.enter_context(tc.tile_pool(name="pp", bufs=2, space="PSUM"))
        pt = ctx.enter_context(tc.tile_pool(name="pt", bufs=3, space="PSUM"))
        po = ctx.enter_context(tc.tile_pool(name="po", bufs=2, space="PSUM"))

        def cpy(eng, out, in_):
            if eng is nc.scalar:
                nc.scalar.copy(out, in_)
            else:
                eng.tensor_copy(out, in_)

        ident = const.tile([128, 128], f32)
        make_identity(nc, ident)
        ones_c = const.tile([128, 1], f32)
        nc.vector.memset(ones_c[:], 1.0)
        cos_sb = const.tile([128, 2, HD], f32)
        sin_sb = const.tile([128, 2, HD], f32)
        mask_sb = const.tile([128, 2, T], f32)
        nc.sync.dma_start(cos_sb[:], cos_d.ap())
        nc.sync.dma_start(sin_sb[:], sin_d.ap())
        nc.sync.dma_start(mask_sb[:], mask_d.ap())

        # new_k/new_v past halves straight from the old cache (DRAM->DRAM)
        nc.sync.dma_start(nk_d.ap()[:, :, 0:PASTVIS, :],
                          pk_d.ap()[:, :, CACHE - PASTVIS:, :])
        nc.sync.dma_start(nv_d.ap()[:, :, 0:PASTVIS, :],
                          pv_d.ap()[:, :, CACHE - PASTVIS:, :])

        # ---- phase B: load x in chunks, transpose to xT [k, kk, tok] ----
        xT = big.tile([128, NK, TOK], f32r, tag="bigT")
        with tc.tile_pool(name="xc", bufs=2) as xcp:
            for tt in range(4):
                x_c = xcp.tile([128, DMODEL], f32, tag="xc")
                nc.sync.dma_start(
                    x_c[:],
                    x_d.ap().rearrange("(tt p) d -> p tt d", p=128)[:, tt, :])
                for kg in range(4):
                    ps = pt.tile([128, 512], f32, tag="pt")
                    for k4 in range(4):
                        kk = 4 * kg + k4
                        nc.tensor.matmul(
                            ps[:, k4 * 128:(k4 + 1) * 128],
                            x_c[:, kk * 128:(kk + 1) * 128], ident[:],
                            is_transpose=True, start=(k4 == 0), stop=(k4 == 3))
                    eng = nc.vector if kg % 2 == 0 else nc.scalar
                    cpy(eng,
                        xT[:, 4 * kg:4 * kg + 4, tt * 128:(tt + 1) * 128],
                        ps[:].rearrange("p (a b) -> p a b", b=128))

        dpool = ctx.enter_context(tc.tile_pool(name="dpool", bufs=2))
        outp = ctx.enter_context(tc.tile_pool(name="outp", bufs=2))

        # ---- phase C: projections (streamed transposed weights) ----
        def run_proj(wname, lhs_src, evict_fn):
            wap = w_d[wname].ap()
            for nch in range(NCH):
                wrows = wrp.tile([128, 2, DMODEL], f32, tag="wrows")
                nc.sync.dma_start(
                    wrows[:],
                    wap[nch * 256:(nch + 1) * 256, :].rearrange(
                        "(nb p) k -> p nb k", p=128))
                wT = wtp.tile([128, NK, 256], f32r, tag="wT")
                for kk in range(NK):
                    ps = pt.tile([128, 512], f32, tag="pt")
                    for nb in range(2):
                        nc.tensor.matmul(
                            ps[:, nb * 128:(nb + 1) * 128],
                            wrows[:, nb, kk * 128:(kk + 1) * 128],
                            ident[:], is_transpose=True,
                            start=(nb == 0), stop=(nb == 1))
                    eng = nc.vector if kk % 2 == 0 else nc.scalar
                    cpy(eng, wT[:, kk, 0:256], ps[:, 0:256])
                for tt in range(4):
                    psm = pp.tile([128, 256], f32, tag="pp")
                    for kk in range(NK):
                        nc.tensor.matmul(
                            psm[:],
                            lhs_src[:, kk, tt * 128:(tt + 1) * 128],
                            wT[:, kk, :],
                            start=(kk == 0), stop=(kk == NK - 1))
                    evict_fn(nch, tt, psm)

        q_tok = sb.tile([128, 4, DMODEL], f32, tag="q_tok")
        k_tok = sb.tile([128, 4, DMODEL], f32, tag="k_tok")
        v_tok = sb.tile([128, 4, H, HD + 1], f32r, tag="v_tok")
        nc.vector.tensor_copy(v_tok[:, :, :, HD:HD + 1],
                              ones_c[:, None, None, :].to_broadcast((128, 4, H, 1)))

        def mk_evict(dst):
            def evict(nch, tt, psm):
                eng = nc.vector if tt % 2 == 0 else nc.scalar
                cpy(eng, dst[:, tt, nch * 256:(nch + 1) * 256], psm[:])
            return evict

        def evict_v(nch, tt, psm):
            eng = nc.vector if tt % 2 == 0 else nc.scalar
            cpy(eng, 
                v_tok[:, tt, nch * 4:(nch + 1) * 4, 0:HD],
                psm[:].rearrange("p (h d) -> p h d", d=HD))

        def rope(tok_tile, engine):
            # pairs (2i, 2i+1) on the free dim:
            # out[2i] = a*cos - b*sin ; out[2i+1] = a*sin + b*cos
            for tt in range(4):
                th = tt % 2
                for hh in range(2):
                    seg = tok_tile[:, tt, hh * 1024:(hh + 1) * 1024]
                    segv = seg.rearrange("p (h i two) -> p h i two", two=2, i=32)
                    t_sh = tmp.tile([128, 1024], f32, tag="t_sh")
                    t_shv = t_sh.rearrange("p (h i two) -> p h i two", two=2, i=32)
                    engine.tensor_copy(t_shv[:, :, :, 0], segv[:, :, :, 1])
                    engine.tensor_copy(t_shv[:, :, :, 1], segv[:, :, :, 0])
                    cosb = cos_sb[:, th, None, :].to_broadcast((128, 16, HD))
                    sinb = sin_sb[:, th, None, :].to_broadcast((128, 16, HD))
                    segh = seg.rearrange("p (h d) -> p h d", d=HD)
                    t_shh = t_sh.rearrange("p (h d) -> p h d", d=HD)
                    t_m = tmp.tile([128, 1024], f32, tag="t_m")
                    t_mh = t_m.rearrange("p (h d) -> p h d", d=HD)
                    engine.tensor_tensor(t_mh[:], segh[:], cosb, OP.mult)
                    engine.tensor_tensor(t_shh[:], t_shh[:], sinb, OP.mult)
                    engine.tensor_tensor(segh[:], t_mh[:], t_shh[:], OP.add)

        run_proj("Wq", xT, mk_evict(q_tok))
        rope(q_tok, nc.vector)
        run_proj("Wk", xT, mk_evict(k_tok))
        for b in range(BLOC):
            for i in range(2):
                nc.sync.dma_start(
                    nk_d.ap()[b, :, PASTVIS + i * 128:PASTVIS + (i + 1) * 128, :]
                    .rearrange("h p d -> p h d"),
                    k_tok[:, 2 * b + i, :].rearrange("p (h d) -> p h d", d=HD))
        rope(k_tok, nc.gpsimd)
        run_proj("Wv", xT, evict_v)
        for b in range(BLOC):
            for i in range(2):
                nc.sync.dma_start(
                    nv_d.ap()[b, :, PASTVIS + i * 128:PASTVIS + (i + 1) * 128, :]
                    .rearrange("h p d -> p h d"),
                    v_tok[:, 2 * b + i, :, 0:HD].bitcast(f32))

        # ---- phase D: attention per (batch, head-pair) ----
        attnT = big.tile([128, NK, TOK], f32r, tag="bigT")  # reuses xT's slot

        def tr2(src0, src1, dst, eng):
            ps = pt.tile([128, 512], f32, tag="pt")
            nc.tensor.matmul(ps[:, 0:128], src0, ident[:],
                             is_transpose=True, start=True, stop=False)
            nc.tensor.matmul(ps[:, 128:256], src1, ident[:],
                             is_transpose=True, start=False, stop=True)
            cpy(eng, dst[:], ps[:, 0:256])

        for b in range(BLOC):
            for g in range(H // 2):
                kp = dpool.tile([128, 2, 2, HD], f32, tag="kp")
                for st in range(2):
                    nc.sync.dma_start(
                        kp[:, st, :, :],
                        pk_d.ap()[b, 2 * g:2 * g + 2,
                                  CACHE - PASTVIS + st * 128:
                                  CACHE - PASTVIS + (st + 1) * 128, :]
                        .rearrange("h p d -> p h d"))
                vx = dpool.tile([128, 2, 2, HD + 1], f32r, tag="vx")
                nc.vector.tensor_copy(vx[:, :, :, HD:HD + 1],
                                      ones_c[:, None, None, :].to_broadcast((128, 2, 2, 1)))
                for st in range(2):
                    nc.gpsimd.dma_start(
                        vx[:, st, :, 0:HD],
                        pv_d.ap()[b, 2 * g:2 * g + 2,
                                  CACHE - PASTVIS + st * 128:
                                  CACHE - PASTVIS + (st + 1) * 128, :]
                        .rearrange("h p d -> p h d"))

                kTp = dpool.tile([128, 256], f32r, tag="kTp")
                tr2(kp[:, 0, :, :], kp[:, 1, :, :], kTp, nc.vector)
                qTg = dpool.tile([128, 256], f32r, tag="qTg")
                tr2(q_tok[:, 2 * b, g * 128:(g + 1) * 128],
                    q_tok[:, 2 * b + 1, g * 128:(g + 1) * 128], qTg, nc.scalar)
                kTc = dpool.tile([128, 256], f32r, tag="kTc")
                tr2(k_tok[:, 2 * b, g * 128:(g + 1) * 128],
                    k_tok[:, 2 * b + 1, g * 128:(g + 1) * 128], kTc, nc.vector)

                for hp in range(2):
                    p0 = 64 * hp
                    psA = pt.tile([128, 512], f32, tag="pt")
                    psB = pt.tile([128, 512], f32, tag="pt")
                    nc.tensor.matmul(psA[:, 0:256], kTp[p0:p0 + 64, 0:128],
                                     qTg[p0:p0 + 64, :], start=True, stop=False)
                    nc.tensor.matmul(psA[:, 256:512], kTp[p0:p0 + 64, 128:256],
                                     qTg[p0:p0 + 64, :], start=False, stop=True)
                    nc.tensor.matmul(psB[:, 0:256], kTc[p0:p0 + 64, 0:128],
                                     qTg[p0:p0 + 64, :], start=True, stop=False)
                    nc.tensor.matmul(psB[:, 256:512], kTc[p0:p0 + 64, 128:256],
                                     qTg[p0:p0 + 64, :], start=False, stop=True)
                    ex = dpool.tile([128, 4, T], f32r, tag="ex")
                    nc.scalar.activation(
                        ex[:, 0:2, :].rearrange("p a b -> p (a b)"), psA[:],
                        AF.Exp, scale=0.125)
                    nc.scalar.activation(
                        ex[:, 2:4, :].rearrange("p a b -> p (a b)"), psB[:],
                        AF.Exp, scale=0.125)
                    nc.vector.tensor_tensor(
                        ex[:, 2, :], ex[:, 2, :], mask_sb[:, 0, :], OP.mult)
                    nc.vector.tensor_tensor(
                        ex[:, 3, :], ex[:, 3, :], mask_sb[:, 1, :], OP.mult)
                    pso = po.tile([128, 256], f32, tag="po")
                    h = 2 * g + hp
                    for j in range(4):
                        lhs = (vx[:, j, hp, :] if j < 2
                               else v_tok[:, 2 * b + (j - 2), h, :])
                        nc.tensor.matmul(pso[0:HD + 1, :], lhs, ex[:, j, :],
                                         start=(j == 0), stop=(j == 3))
                    rcp = dpool.tile([1, 256], f32, tag="rcp")
                    nc.vector.reciprocal(rcp[:], pso[HD:HD + 1, :])
                    rcpb = dpool.tile([64, 256], f32, tag="rcpb")
                    nc.gpsimd.partition_broadcast(rcpb[:], rcp[:])
                    nc.vector.tensor_tensor(
                        attnT[p0:p0 + 64, g, b * 256:(b + 1) * 256],
                        pso[0:HD, :], rcpb[:], OP.mult)

        # ---- phase E: output projection ----
        osb = {}

        def evict_o(nch, tt, psm):
            if tt == 0:
                osb[0] = outp.tile([128, 4, 256], f32, tag="osb", name="osb")
            eng = nc.vector if tt % 2 == 0 else nc.scalar
            cpy(eng, osb[0][:, tt, :], psm[:])
            if tt == 3:
                nc.sync.dma_start(
                    out_d.ap().rearrange("(tt p) d -> p tt d", p=128)
                    [:, :, nch * 256:(nch + 1) * 256],
                    osb[0][:])

        run_proj("Wo", attnT, evict_o)

    nc.compile()
    return nc


def _host_tables(pos_offset):
    inv = 1.0 / (THETA ** (np.arange(0, HD, 2, dtype=np.float64) / HD))  # [32]
    t = np.arange(pos_offset, pos_offset + T, dtype=np.float64)
    ang = t[:, None] * inv[None, :]                                      # [256, 32]
    cos = np.cos(ang)
    sin = np.sin(ang)
    cos_t = np.zeros((128, 2, HD), np.float32)
    sin_t = np.zeros((128, 2, HD), np.float32)
    for th in range(2):
        c = cos[th * 128:(th + 1) * 128]
        s = sin[th * 128:(th + 1) * 128]
        cos_t[:, th, 0::2] = c
        cos_t[:, th, 1::2] = c
        sin_t[:, th, 0::2] = -s
        sin_t[:, th, 1::2] = s
    mask = np.zeros((128, 2, T), np.float32)
    sl = np.arange(128)[:, None]
    tpos = np.arange(T)[None, :]
    mask[:, 0, :] = (tpos >= sl).astype(np.float32)
    mask[:, 1, :] = (tpos >= 128 + sl).astype(np.float32)
    return cos_t, sin_t, mask


def kernel(x, past_k, past_v, Wq, Wk, Wv, Wo, pos_offset):
    from concourse.bass_utils import run_bass_kernel_spmd

    if "nc" not in _cache:
        _cache["nc"] = _build_nc()
    nc = _cache["nc"]

    cos_t, sin_t, mask = _host_tables(int(pos_offset))
    x = np.ascontiguousarray(x, dtype=np.float32)
    past_k = np.ascontiguousarray(past_k, dtype=np.float32)
    past_v = np.ascontiguousarray(past_v, dtype=np.float32)
    common = {
        "Wq": np.ascontiguousarray(Wq, np.float32),
        "Wk": np.ascontiguousarray(Wk, np.float32),
        "Wv": np.ascontiguousarray(Wv, np.float32),
        "Wo": np.ascontiguousarray(Wo, np.float32),
        "cos_t": cos_t, "sinsgn_t": sin_t, "mask_t": mask,
    }
    in_maps = []
    for c in range(NCORES):
        sl = slice(c * BLOC, (c + 1) * BLOC)
        in_maps.append({
            "x": np.ascontiguousarray(x[sl].reshape(TOK, DMODEL)),
            "past_k": np.ascontiguousarray(past_k[sl]),
            "past_v": np.ascontiguousarray(past_v[sl]),
            **common,
        })

    res = run_bass_kernel_spmd(nc, in_maps, core_ids=list(range(NCORES)))
    out = np.concatenate(
        [r["out"].reshape(BLOC, T, DMODEL) for r in res.results], axis=0)
    new_k = np.concatenate([r["new_k"] for r in res.results], axis=0)
    new_v = np.concatenate([r["new_v"] for r in res.results], axis=0)
    return out, new_k, new_v
